# revision 48
# baseline (speedup 1.0000x reference)
"""ARMA GNN (2-layer, K=2 stacks) Trainium2 kernel.

v9 "dense expanded layer-1 + identity rounds":
  * Layer 1 does NO gathers at all: the host expands the (dinv-scaled)
    source features into a dense per-core stream in transposed round
    layout [feature, round*128+slot], where round r of window w holds the
    r-th in-edge of each of the window's 128 targets (zeros past a
    target's degree).  Targets are DEGREE-SORTED per core so rounds pad
    only ~2%.  On device each round is accumulated into PSUM agg.T with a
    single matmul against a constant fp16 identity (lhsT never changes).
  * Layer 2 = v8.1 machinery: per-edge SWDGE gathers of half-rows
    (elem 64 on a 256B-stride table), host-built fp8 S matrices streamed
    from HBM, chunk-aligned A/B passes overlapping the AllGather.
  * Output rows come back window-major in degree-sorted order; the host
    applies the inverse permutation.

kernel(**inputs) takes the FULL problem inputs and returns the FULL output.
"""

import sys

sys.path.insert(0, "/opt/trn_rl_repo")

from contextlib import ExitStack

import numpy as np

P = 128


class Cfg:
    def __init__(self, N, NC, SHARD, B0, WB=2, SUPER=4,
                 FIN=128, HID=64, FOUT=64, K=2, SP=False, SDT="f8"):
        self.N, self.NC, self.SHARD, self.B0 = N, NC, SHARD, B0
        self.WB, self.SUPER = WB, SUPER
        self.FIN, self.HID, self.FOUT, self.K = FIN, HID, FOUT, K
        self.SP, self.SDT = SP, SDT
        self.NSTAR = NC * SHARD
        self.W = SHARD // P
        self.HALFA = B0
        self.HALFB = self.NSTAR - B0
        assert B0 % (NC * P) == 0 and SHARD % P == 0
        assert self.HALFA <= 32768 and self.HALFB <= 32768
        assert N > B0 and N < self.NSTAR
        assert K * HID == 128 and K * FOUT == 128 and FIN == 128
        self.wA = B0 // (NC * P)
        self.wB = self.W - self.wA
        self.batches = [tuple(range(b, min(b + WB, self.W)))
                        for b in range(0, self.W, WB)]
        self.sbatches = [self.batches[i:i + SUPER]
                         for i in range(0, len(self.batches), SUPER)]

        def split(n, fracs):
            sizes, rem = [], n
            for f in fracs:
                if rem <= 0:
                    break
                s = max(1, min(rem, round(n * f)))
                sizes.append(s)
                rem -= s
            if rem:
                sizes.append(rem)
            return sizes

        sizes = split(self.wA, (0.21, 0.34)) + [self.wB]
        self.cc_chunks = []
        w0 = 0
        for sz in sizes:
            self.cc_chunks.append((w0, w0 + sz))
            w0 += sz
        assert w0 == self.W
        assert any(w1 == self.wA for (_, w1) in self.cc_chunks)
        self.chunk_base = []
        b = 0
        for (w0, w1) in self.cc_chunks:
            self.chunk_base.append(b)
            b += NC * (w1 - w0) * P
        assert b == self.NSTAR

    def perm_pos(self, pos):
        """degree-rank position (core-major) -> chunk-major table row."""
        pos = np.asarray(pos)
        r, p = pos // self.SHARD, pos % self.SHARD
        w = p // P
        prow = np.zeros_like(pos)
        for q, (w0, w1) in enumerate(self.cc_chunks):
            m = (w >= w0) & (w < w1)
            prow = np.where(
                m, self.chunk_base[q] + r * (w1 - w0) * P + (p - w0 * P), prow)
        return prow


REAL = dict(N=50000, NC=8, SHARD=6272, B0=29696)


def _s_np_dtype(sdt):
    if sdt == "f8":
        import ml_dtypes
        return ml_dtypes.float8_e4m3fn
    return np.float16


# --------------------------------------------------------------------------
# host preprocessing
# --------------------------------------------------------------------------
def _preprocess(c: Cfg, x, edge_index, init_w1, root_w1, b1, init_w2, root_w2, b2):
    N, NC, SHARD = c.N, c.NC, c.SHARD
    row = np.asarray(edge_index[0]).astype(np.int64)
    col = np.asarray(edge_index[1]).astype(np.int64)
    x = np.asarray(x, dtype=np.float32)

    deg_n = np.bincount(col, minlength=N).astype(np.int64)
    deg = np.zeros(c.NSTAR, np.int64)
    deg[:N] = deg_n
    dinv_full = np.where(deg > 0, deg.astype(np.float64) ** -0.5, 0.0
                         ).astype(np.float32)

    # per-core degree-sorted target order; global position space
    tsort = np.zeros((NC, SHARD), np.int64)   # position -> local node
    tpos = np.zeros((NC, SHARD), np.int64)    # local node -> position
    for cc in range(NC):
        dg = deg[cc * SHARD:(cc + 1) * SHARD]
        ts = np.argsort(-dg, kind="stable")
        tsort[cc] = ts
        tpos[cc, ts] = np.arange(SHARD)
    # node id -> table row (chunk-major over degree-rank positions)
    pos_of_node = np.empty(c.NSTAR, np.int64)
    for cc in range(NC):
        pos_of_node[cc * SHARD:(cc + 1) * SHARD] = cc * SHARD + tpos[cc]
    prow_of_node = c.perm_pos(pos_of_node)
    prow_all = prow_of_node[row]

    xs_scaled = (x * dinv_full[:N, None]).astype(np.float16)  # [N,128]

    # per-core layer-1 round counts (degree-sorted => max = first in window)
    Rw = np.zeros((NC, c.W), np.int64)
    for cc in range(NC):
        dg = deg[cc * SHARD:(cc + 1) * SHARD][tsort[cc]]
        Rw[cc] = np.maximum(1, dg.reshape(c.W, P).max(axis=1))
    NR = [int(Rw[:, w].max()) for w in range(c.W)]
    NRtot = sum(NR)
    rbase = np.concatenate([[0], np.cumsum(NR)]).astype(np.int64)

    # pass 1: per-core sorted+deduped layer-2 streams and unique counts
    percore = []
    cntA = np.zeros((NC, c.W), np.int64)
    cntB = np.zeros((NC, c.W), np.int64)
    for cc in range(NC):
        base = cc * SHARD
        m = (col >= base) & (col < base + SHARD)
        ec = (col[m] - base).astype(np.int64)
        es = prow_all[m]
        wslot = tpos[cc][ec]               # degree-sorted (window, slot)
        half = (es >= c.B0).astype(np.int64)
        key = (wslot >> 7) * 2 + half
        order = np.lexsort((es, key))
        ek, ee, ecol = key[order], es[order], (wslot & 127)[order]
        src_l = np.asarray(row[m])[order]
        ws_l = wslot[order]
        new_group = np.empty(len(ek), bool)
        if len(ek):
            new_group[0] = True
            new_group[1:] = (ek[1:] != ek[:-1]) | (ee[1:] != ee[:-1])
        uid = np.cumsum(new_group) - 1
        u_key = ek[new_group]
        u_es = ee[new_group]
        gstart = np.searchsorted(u_key, np.arange(2 * c.W + 1))
        posu = uid - gstart[ek]
        cnt = gstart[1:] - gstart[:-1]
        cntA[cc] = cnt[0::2]
        cntB[cc] = cnt[1::2]
        percore.append((ek, ecol, posu, u_key, u_es, gstart, src_l, ws_l))

    NBA = [max(1, int(-(-cntA[:, w].max() // P))) for w in range(c.W)]
    NBB = [max(1, int(-(-cntB[:, w].max() // P))) for w in range(c.W)]
    NBAtot, NBBtot = sum(NBA), sum(NBB)
    blkA_base = np.concatenate([[0], np.cumsum(NBA)]).astype(np.int64)
    blkB_base = np.concatenate([[0], np.cumsum(NBB)]).astype(np.int64)
    sdt = _s_np_dtype(c.SDT)

    def build_core(cc):
        ek, ecol, posu, u_key, u_es, gstart, src_l, ws_l = percore[cc]
        w_e, h_e = ek >> 1, ek & 1
        outs = []
        for h, NB_base, NBtot in ((0, blkA_base, NBAtot), (1, blkB_base, NBBtot)):
            sel = h_e == h
            g = NB_base[w_e[sel]] + posu[sel] // P
            part = posu[sel] % P
            flat = part * (NBtot * P) + g * P + ecol[sel]
            S = np.bincount(flat, minlength=P * NBtot * P).astype(np.float32)
            outs.append(np.ascontiguousarray(
                S.reshape(P, NBtot * P).astype(sdt)))
        toksA, toksB = [], []
        for w in range(c.W):
            for h, toks, NB in ((0, toksA, NBA), (1, toksB, NBB)):
                gi = 2 * w + h
                lo, hi = gstart[gi], gstart[gi + 1]
                k = hi - lo
                t = np.zeros(NB[w] * P, np.int64)
                t[:k] = u_es[lo:hi] - (c.B0 if h else 0)
                toks.append(t)

        def mk_idx(toks):
            toks = np.concatenate(toks)
            L = len(toks)
            return np.ascontiguousarray(
                np.tile(toks.reshape(L // 16, 16).T.astype(np.int16), (8, 1)))

        # layer-1 expanded stream [128 f, NRtot*128] f16 in round layout
        strm = np.zeros((P, NRtot * P), np.float16)
        o2 = np.argsort(ws_l, kind="stable")       # edges by (window,slot)
        ws_s = ws_l[o2]
        src_s = src_l[o2]
        first = np.searchsorted(ws_s, np.arange(SHARD + 1))
        r_e = np.arange(len(ws_s)) - first[ws_s]   # per-target round index
        w_s = ws_s >> 7
        s_s = ws_s & 127
        cols = (rbase[w_s] + r_e) * P + s_s
        strm[:, cols] = xs_scaled[src_s].T
        return outs[0], outs[1], mk_idx(toksA), mk_idx(toksB), strm

    xpad = np.zeros((c.NSTAR, c.FIN), np.float32)
    xpad[:N] = x

    def cat2(w, dt):
        w = np.asarray(w, dtype=np.float32)
        return np.ascontiguousarray(np.concatenate([w[0], w[1]], axis=1).astype(dt))

    w1cat = cat2(init_w1, np.float32)            # [128,128] f32
    w2cat = cat2(init_w2, np.float16)            # [64,128]  f16
    rw1c = cat2(0.5 * np.asarray(root_w1, np.float32), np.float16)
    rw2c = cat2(0.5 * np.asarray(root_w2, np.float32), np.float32)
    b1 = np.asarray(b1, dtype=np.float32)
    b2 = np.asarray(b2, dtype=np.float32)
    b1b = np.ascontiguousarray(
        np.tile(0.5 * np.concatenate([b1[0], b1[1]]), (P, 4)))
    b2b = np.ascontiguousarray(
        np.tile(0.5 * np.concatenate([b2[0], b2[1]]), (P, 4)))

    in_maps = []
    for cc in range(NC):
        base = cc * SHARD
        sA, sB, idxA, idxB, strm = build_core(cc)
        ts = tsort[cc]
        dloc = dinv_full[base:base + SHARD][ts]
        dinvo = 0.5 * dloc.reshape(c.W, P).T
        dinvt = dloc.reshape(c.W, P).T
        in_maps.append({
            "strm": strm,
            "xTow": np.ascontiguousarray(
                xpad[base + ts].T.astype(np.float16)),
            "w1cat": w1cat, "rw1c": rw1c, "w2cat": w2cat, "rw2c": rw2c,
            "b1b": b1b, "b2b": b2b,
            "dinvo": np.ascontiguousarray(dinvo.astype(np.float32)),
            "dinvt": np.ascontiguousarray(dinvt.astype(np.float32)),
            "idxA": idxA, "idxB": idxB,
            "sA": sA, "sB": sB,
        })
    return in_maps, NBA, NBB, NR, tsort


# --------------------------------------------------------------------------
# device program
# --------------------------------------------------------------------------
def _build_program(c: Cfg, NBA, NBB, NR):
    import concourse.tile as tile
    from concourse import bacc, mybir
    from concourse.masks import make_identity

    f32 = mybir.dt.float32
    f16 = mybir.dt.float16
    i16 = mybir.dt.int16
    fS = mybir.dt.float8e4 if c.SDT == "f8" else f16
    AL = mybir.AluOpType
    AF = mybir.ActivationFunctionType

    NBAtot, NBBtot = sum(NBA), sum(NBB)
    NRtot = sum(NR)
    LA, LB = NBAtot * P, NBBtot * P
    rbase = np.concatenate([[0], np.cumsum(NR)]).astype(np.int64)

    nc = bacc.Bacc("TRN2", target_bir_lowering=False, debug=False,
                   num_devices=c.NC, num_swdge_queues=4)
    qrr = [0]

    def din(name, shape, dt=f32):
        return nc.dram_tensor(name, shape, dt, kind="ExternalInput")

    strmd = din("strm", [P, NRtot * P], f16)     # layer-1 round stream
    xTow = din("xTow", [P, c.SHARD], f16)
    w1cat = din("w1cat", [P, 128], f32)
    rw1c = din("rw1c", [P, 128], f16)
    w2cat = din("w2cat", [64, 128], f16)
    rw2c = din("rw2c", [64, 128], f32)
    b1b = din("b1b", [P, 512]); b2b = din("b2b", [P, 512])
    dinvo = din("dinvo", [P, c.W])
    dinvt = din("dinvt", [P, c.W])
    idxA = din("idxA", [P, LA // 16], i16)
    idxB = din("idxB", [P, LB // 16], i16)
    sAd = din("sA", [P, NBAtot * P], fS)
    sBd = din("sB", [P, NBBtot * P], fS)
    yt = nc.dram_tensor("yt", [c.SHARD, 64], f32, kind="ExternalOutput")

    ccpad = nc.dram_tensor("ccpad", [c.SHARD, 128], f16)
    ccout = nc.dram_tensor("ccout", [c.NSTAR, 128], f16, addr_space="Shared")

    with tile.TileContext(nc) as tc, ExitStack() as ctx:
        cpool = ctx.enter_context(tc.tile_pool(name="consts", bufs=1))
        xtp = ctx.enter_context(tc.tile_pool(name="xtp", bufs=3))
        stp = ctx.enter_context(tc.tile_pool(name="stp", bufs=6))
        gth = ctx.enter_context(tc.tile_pool(name="gth", bufs=10))
        sgp = ctx.enter_context(tc.tile_pool(name="sgp", bufs=6))
        idxp = ctx.enter_context(tc.tile_pool(name="idxp", bufs=3))
        epi = ctx.enter_context(tc.tile_pool(name="epi", bufs=3))
        big = ctx.enter_context(tc.tile_pool(name="big", bufs=1))
        psA = ctx.enter_context(tc.tile_pool(name="psA", bufs=1, space="PSUM"))
        psB = ctx.enter_context(tc.tile_pool(name="psB", bufs=5, space="PSUM"))
        psC = ctx.enter_context(tc.tile_pool(name="psC", bufs=2, space="PSUM"))

        ident = cpool.tile([P, P], f32, tag="ident")
        make_identity(nc, ident[:])
        ident16 = cpool.tile([P, P], f16, tag="ident16")
        nc.vector.tensor_copy(ident16[:], ident[:])

        def load_const(dram, shape, tag, dt=f32):
            t = cpool.tile(shape, dt, tag=tag)
            nc.sync.dma_start(t[:], dram[:, :])
            return t

        w1_s = load_const(w1cat, [P, 128], "w1")
        rw1_s = load_const(rw1c, [P, 128], "rw1", f16)
        w2_s = load_const(w2cat, [64, 128], "w2", f16)
        rw2_s = load_const(rw2c, [64, 128], "rw2")
        b1_s = load_const(b1b, [P, 512], "b1")
        b2_s = load_const(b2b, [P, 512], "b2")
        dinvo_s = load_const(dinvo, [P, c.W], "dinvo")
        dinvt_s = load_const(dinvt, [P, c.W], "dinvt")

        def gather_call(tab_ap, ix_t, l0, nblk):
            g_t = gth.tile([P, nblk * 64], f16, tag="gath")
            out_ap = g_t[:].rearrange("p (b f) -> p b f", f=64)
            idxs_ap = ix_t[:, l0 // 16:(l0 + nblk * P) // 16]
            eng = nc.gpsimd
            _in_ap = eng.lower_ap_dma(tab_ap, for_custom_bir_dma=True)
            _idxs_ap = eng.lower_ap(idxs_ap)
            _out_ap = eng.lower_ap(out_ap)
            eng.add_instruction(
                mybir.InstDMAGatherAnt(
                    name=eng.bass.get_next_instruction_name(),
                    ins=[*_in_ap, _idxs_ap,
                         eng.lower_val_access(eng.to_reg(nblk * P))],
                    outs=[_out_ap],
                    transpose=False,
                    num_idxs=nblk * P,
                    elem_size=64,
                    stride_bytes_256=1,
                    gen_mode=0,
                    single_packet=c.SP,
                    queue_num=qrr[0] % 4,
                    sbuf_tokens_per_rank=0,
                    sbuf_free_dim_per_rank=0,
                    sbuf_free_dim_pad_per_rank=0,
                    sbuf_byte_offset=0,
                ))
            qrr[0] += 1
            return g_t

        # ---- queue warmup: tiny gather per SWDGE queue, overlaps prolog ----
        with nc.named_scope("warm"):
            wix = idxp.tile([P, 8], i16, tag="ixA")
            nc.sync.dma_start(wix[:], idxA[:, 0:8])
            for q in range(4):
                gather_call(ccout[0:c.HALFA, 0:64], wix, 0, 1)

        # ---- prolog: root1 (bias adds batched 4 windows per op) ----
        with nc.named_scope("prolog"):
            root1 = big.tile([P, c.SHARD], f32, tag="root")
            i = 0
            while i < c.W:
                n = min(8, c.W - i)
                xp = xtp.tile([P, 8 * 128], f16, tag="xtp")
                nc.sync.dma_start(xp[:, :n * 128], xTow[:, i * P:(i + n) * P])
                j = 0
                while j < n:
                    g = min(4, n - j)
                    ps = psA.tile([P, 512], f32, tag="grp")
                    for k in range(g):
                        nc.tensor.matmul(
                            out=ps[:, k * 128:(k + 1) * 128],
                            lhsT=xp[:, (j + k) * 128:(j + k + 1) * 128],
                            rhs=rw1_s[:], start=True, stop=True)
                    nc.vector.tensor_tensor(
                        out=root1[:, (i + j) * 128:(i + j + g) * 128],
                        in0=ps[:, :g * 128], in1=b1_s[:, :g * 128], op=AL.add)
                    j += g
                i += n

        def s_load(sd, blk0, nblk):
            s_t = sgp.tile([P, nblk * 128], fS, tag="sg")
            nc.sync.dma_start(s_t[:], sd[:, blk0 * 128:(blk0 + nblk) * 128])
            return s_t

        # prefetch layer-2 pass feeds (sync queue, ahead of the big
        # stream loads so they land while layer 1 runs)
        sb0A = sum(NBA[w] for b in c.sbatches[0] for w in b) * P
        pre_ixA = idxp.tile([P, sb0A // 16], i16, tag="ixA")
        nc.sync.dma_start(pre_ixA[:], idxA[:, 0:sb0A // 16])
        sb0B = sum(NBB[w] for b in c.sbatches[0] for w in b) * P
        pre_ixB = idxp.tile([P, sb0B // 16], i16, tag="ixB")
        nc.sync.dma_start(pre_ixB[:], idxB[:, 0:sb0B // 16])
        pre_sA = [s_load(sAd, 0, sum(NBA[w] for w in c.batches[0])),
                  s_load(sAd, sum(NBA[w] for w in c.batches[0]),
                         sum(NBA[w] for w in c.batches[1]))]

        # ---- layer 1: dense round stream + identity accumulation ----
        h1n = big.tile([P, c.W * 64], f16, tag="ht")
        pending = []
        chunk_iter = iter(range(len(c.cc_chunks)))
        next_q = next(chunk_iter)

        def issue_cc(q):
            w0, w1 = c.cc_chunks[q]
            b0 = c.chunk_base[q]
            nc.gpsimd.collective_compute(
                "AllGather", AL.bypass,
                replica_groups=[list(range(c.NC))],
                ins=[ccpad[w0 * P:w1 * P, :].opt()],
                outs=[ccout[b0:b0 + c.NC * (w1 - w0) * P, :].opt()])

        def on_window1(w):
            nonlocal next_q
            while pending and w >= pending[0][1]:
                issue_cc(pending.pop(0)[0])
            sc = epi.tile([P, 64], f16, tag="sc")
            nc.scalar.mul(sc[:], h1n[:, w * 64:(w + 1) * 64],
                          dinvt_s[:, w:w + 1])
            nc.scalar.dma_start(
                ccpad[w * P:(w + 1) * P, 0:64]
                .rearrange("(k p) f -> p k f", p=P),
                sc[:].rearrange("p (k f) -> p k f", k=1))
            if next_q is not None and w == c.cc_chunks[next_q][1] - 1:
                pending.append((next_q, w + c.WB))
                next_q = next(chunk_iter, None)

        RT = 16  # rounds per stream tile
        n_st = -(-NRtot // RT)
        st_tiles = {}

        def ensure_st(k):
            if k not in st_tiles and k < n_st:
                t = stp.tile([P, RT * 128], f16, tag="strm")
                c0 = k * RT
                ncols = (min(RT, NRtot - c0)) * 128
                nc.sync.dma_start(t[:, :ncols],
                                  strmd[:, c0 * 128:c0 * 128 + ncols])
                st_tiles[k] = t

        with nc.named_scope("layer1"):
            for k in range(3):
                ensure_st(k)
            for batch in c.batches:
                for w in batch:
                    g0, g1 = int(rbase[w]), int(rbase[w] + NR[w])
                    for k in range(g0 // RT, (g1 - 1) // RT + 3):
                        ensure_st(k)
                    pw = psB.tile([P, 128], f32, tag="pw")
                    for r in range(NR[w]):
                        g = g0 + r
                        nc.tensor.matmul(
                            out=pw[:],
                            lhsT=ident16[:],
                            rhs=st_tiles[g // RT][:, (g % RT) * 128:
                                                  (g % RT + 1) * 128],
                            start=(r == 0), stop=(r == NR[w] - 1))
                    utc = epi.tile([P, 128], f32, tag="utc")
                    nc.scalar.copy(utc[:], pw[:])
                    pw2 = psC.tile([P, 128], f32, tag="pw2")
                    nc.tensor.matmul(out=pw2[:], lhsT=utc[:],
                                     rhs=w1_s[:], start=True, stop=True)
                    t2 = epi.tile([P, 128], f32, tag="t2")
                    nc.vector.scalar_tensor_tensor(
                        out=t2[:], in0=pw2[:], scalar=dinvo_s[:, w:w + 1],
                        in1=root1[:, w * 128:(w + 1) * 128],
                        op0=AL.mult, op1=AL.add)
                    t3 = epi.tile([P, 128], f32, tag="t3")
                    nc.scalar.activation(t3[:], t2[:], AF.Relu)
                    nc.vector.tensor_tensor(
                        out=h1n[:, w * 64:(w + 1) * 64],
                        in0=t3[:, :64], in1=t3[:, 64:], op=AL.add)
                    on_window1(w)

        with nc.named_scope("cc"):
            for q, _ in pending:
                issue_cc(q)

        # ---- layer 2 pass A ----
        aggA16 = big.tile([64, c.W * 128], f16, tag="aggA")
        root2 = big.tile([P, c.SHARD], f32, tag="root")  # aliases root1

        def root2_group(j, g):
            ps = psA.tile([P, 512], f32, tag="grp")
            for k in range(g):
                u2 = epi.tile([P, 64], f32, tag="u2")
                nc.scalar.copy(u2[:], h1n[:, (j + k) * 64:(j + k + 1) * 64])
                tp_ = psC.tile([P, 128], f32, tag="pw2")
                nc.tensor.transpose(out=tp_[:64, :], in_=u2[:],
                                    identity=ident[:])
                hl = epi.tile([64, 128], f32, tag="hl")
                nc.scalar.copy(hl[:], tp_[:64, :])
                nc.tensor.matmul(out=ps[:, k * 128:(k + 1) * 128],
                                 lhsT=hl[:], rhs=rw2_s[:],
                                 start=True, stop=True)
            nc.vector.tensor_tensor(
                out=root2[:, j * 128:(j + g) * 128],
                in0=ps[:, :g * 128], in1=b2_s[:, :g * 128], op=AL.add)

        with nc.named_scope("l2passA"):
            blkA = 0
            tokA = 0
            r2done = 0
            for si, sb in enumerate(c.sbatches):
                sbA = sum(NBA[w] for b in sb for w in b) * P
                if si == 0:
                    ixA = pre_ixA
                else:
                    ixA = idxp.tile([P, sbA // 16], i16, tag="ixA")
                    nc.sync.dma_start(ixA[:],
                                      idxA[:, tokA // 16:(tokA + sbA) // 16])
                lA = 0
                for batch in sb:
                    nA = sum(NBA[w] for w in batch)
                    gA = gather_call(ccout[0:c.HALFA, 0:64], ixA, lA, nA)
                    sA_t = pre_sA.pop(0) if pre_sA else s_load(sAd, blkA, nA)
                    lA += nA * P
                    oA = 0
                    for w in batch:
                        pw = psB.tile([P, 128], f32, tag="pw")
                        for j in range(NBA[w]):
                            b = oA + j
                            nc.tensor.matmul(
                                out=pw[:64, :],
                                lhsT=gA[:, b * 64:(b + 1) * 64],
                                rhs=sA_t[:, b * 128:(b + 1) * 128],
                                start=(j == 0), stop=(j == NBA[w] - 1))
                        oA += NBA[w]
                        nc.scalar.copy(aggA16[:, w * 128:(w + 1) * 128],
                                       pw[:64, :])
                        while r2done <= w - 3:
                            g = min(4, c.W - r2done)
                            root2_group(r2done, g)
                            r2done += g
                    blkA += nA
                tokA += sbA
            while r2done < c.W:
                g = min(4, c.W - r2done)
                root2_group(r2done, g)
                r2done += g

        # ---- layer 2 pass B ----
        with nc.named_scope("l2passB"):
            blkB = 0
            tokB = 0
            for si, sb in enumerate(c.sbatches):
                sbB = sum(NBB[w] for b in sb for w in b) * P
                if si == 0:
                    ixB = pre_ixB
                else:
                    ixB = idxp.tile([P, sbB // 16], i16, tag="ixB")
                    nc.sync.dma_start(ixB[:],
                                      idxB[:, tokB // 16:(tokB + sbB) // 16])
                lB = 0
                for batch in sb:
                    nB = sum(NBB[w] for w in batch)
                    gB = gather_call(ccout[c.HALFA:c.NSTAR, 0:64], ixB, lB, nB)
                    sB_t = s_load(sBd, blkB, nB)
                    lB += nB * P
                    oB = 0
                    for w in batch:
                        pw = psB.tile([P, 128], f32, tag="pw")
                        for j in range(NBB[w]):
                            b = oB + j
                            nc.tensor.matmul(
                                out=pw[:64, :],
                                lhsT=gB[:, b * 64:(b + 1) * 64],
                                rhs=sB_t[:, b * 128:(b + 1) * 128],
                                start=(j == 0), stop=(j == NBB[w] - 1))
                        oB += NBB[w]
                        utcB = epi.tile([64, 128], f16, tag="utcB")
                        nc.scalar.copy(utcB[:], pw[:64, :])
                        pw2 = psC.tile([P, 128], f32, tag="pw2")
                        nc.tensor.matmul(
                            out=pw2[:], lhsT=aggA16[:, w * 128:(w + 1) * 128],
                            rhs=w2_s[:], start=True, stop=False)
                        nc.tensor.matmul(
                            out=pw2[:], lhsT=utcB[:],
                            rhs=w2_s[:], start=False, stop=True)
                        t2 = epi.tile([P, 128], f32, tag="t2")
                        nc.vector.scalar_tensor_tensor(
                            out=t2[:], in0=pw2[:], scalar=dinvo_s[:, w:w + 1],
                            in1=root2[:, w * 128:(w + 1) * 128],
                            op0=AL.mult, op1=AL.add)
                        t3 = epi.tile([P, 128], f32, tag="t3")
                        nc.scalar.activation(t3[:], t2[:], AF.Relu)
                        yv = epi.tile([P, 64], f32, tag="yv")
                        nc.vector.tensor_tensor(
                            out=yv[:], in0=t3[:, :64], in1=t3[:, 64:],
                            op=AL.add)
                        nc.scalar.dma_start(
                            yt[w * P:(w + 1) * P, :]
                            .rearrange("(k p) f -> p k f", p=P),
                            yv[:].rearrange("p (k f) -> p k f", k=1))
                    blkB += nB
                tokB += sbB

    nc.compile()
    return nc


_cache = {}


def prepare(inputs, cfg_kw=None):
    c = Cfg(**(cfg_kw or REAL))
    in_maps, NBA, NBB, NR, tsort = _preprocess(c, **inputs)
    key = (tuple(sorted((cfg_kw or REAL).items())), tuple(NBA), tuple(NBB),
           tuple(NR))
    if key not in _cache:
        _cache[key] = _build_program(c, NBA, NBB, NR)
    return c, _cache[key], in_maps, tsort


def kernel(x, edge_index, init_w1, root_w1, b1, init_w2, root_w2, b2,
           _trace=False, _cfg=None):
    from concourse import bass_utils
    inputs = dict(x=np.asarray(x), edge_index=np.asarray(edge_index),
                  init_w1=np.asarray(init_w1), root_w1=np.asarray(root_w1),
                  b1=np.asarray(b1), init_w2=np.asarray(init_w2),
                  root_w2=np.asarray(root_w2), b2=np.asarray(b2))
    c, nc, in_maps, tsort = prepare(inputs, _cfg)
    res = bass_utils.run_bass_kernel_spmd(
        nc, in_maps, core_ids=list(range(c.NC)), trace=_trace)
    out = np.empty((c.NSTAR, 64), np.float32)
    for cc in range(c.NC):
        blk = out[cc * c.SHARD:(cc + 1) * c.SHARD]
        blk[tsort[cc]] = res.results[cc]["yt"]
    out = out[:c.N]
    if _trace:
        kernel._last = res
    return np.ascontiguousarray(out.astype(np.float32))
